# revision 23
# baseline (speedup 1.0000x reference)
"""DNC (Differentiable Neural Computer) scan kernel for Trainium2 — v2.

Sharding: data-parallel over batch B=32 across 8 NeuronCores (4 examples/core).
Per core, a fully-unrolled T-step scan with N=128 memory slots on the SBUF
partition dim.

v2 restructure vs v1 baseline (3.08ms):
- GpSimd only does the one unavoidable row-broadcast (ubc) per step; the
  colsum-broadcast reductions (sum ww / softmax denmodels) are ones-matmuls on
  the PE; both gpsimd pow ops are replaced by DVE bit-hacks:
    rsqrt via 0x5F3759DF + 1 Newton step (rel err < 0.2%)
    softplus via exponent-extraction log2 + cubic minimax poly (abs err < 1e-3)
  so cw = exp(ln2 * cos * log2(1+e^z)) with the final Exp on ScalarE.
- Sigmoids are never materialized: tanh(x/2) identities are folded into the
  consumers (h and c are stored doubled; W_hh/W_if/W_out[h] pre-halved).
- The allocation masked-product uses min(1, [u~_j >= u~_i] + u~_j) (one STT
  per example + one min) instead of compare/mult/bias.
- link and mem state live in bf16 (DVE 2x mode); fw/bw read them directly
  (no psum->bf16 copies of link).
- Next step's h-part gate GEMM is emitted at the end of each step and the
  alloc chain right after the usage update, so the Tile scheduler overlaps
  them with the link/read tail.
"""

import sys
from contextlib import ExitStack

import ml_dtypes
import numpy as np

BFNP = ml_dtypes.bfloat16

sys.path.insert(0, "/opt/trn_rl_repo")

import concourse.bass as bass  # noqa: E402
import concourse.bacc as bacc  # noqa: E402
import concourse.bass_isa as bass_isa  # noqa: E402
import concourse.tile as tile  # noqa: E402
from concourse import mybir  # noqa: E402
from concourse import bass_utils  # noqa: E402
from concourse.hw_specs import get_activation_tables  # noqa: E402

AF = mybir.ActivationFunctionType
OP = mybir.AluOpType
AX = mybir.AxisListType
RED = bass_isa.ReduceOp
F32 = mybir.dt.float32
BF16 = mybir.dt.bfloat16
I32 = mybir.dt.int32

B, T, I, O, H = 32, 64, 64, 64, 512
N, W = 128, 64
IF = 3 * W + 6
NCORES = 8
BB = B // NCORES  # 4 examples per core
NMB = 16          # gate-dim m-blocks (2048/128)
NKH = 4           # h k-chunks (512/128)

LN2 = 0.6931471805599453
# log2(m) cubic on [1,2]:  ((C3*m + C2)*m + C1)*m + C0
C3, C2, C1, C0 = 0.15544586, -1.03925816, 3.02947821, -2.14494063
# quadratic variant (max abs err 5.6e-3, ~0.4% on cw -- inside tolerance)
C2Q, C1Q, C0Q = -0.33975181, 2.00936479, -1.66403002


class _Bacc(bacc.Bacc):
    """Single activation-table load: keep exp_and_others (Exp/Tanh/Square/
    Identity/Copy), empty the rest so no mid-kernel table reloads happen."""

    def insert_act_table_loads(self):
        has_activation = any(
            isinstance(i, mybir.InstActivation)
            for b in self.main_func.blocks
            for i in b.instructions
        )
        if not has_activation:
            return
        tables = [
            (name, (s if name == "exp_and_others" else type(s)()))
            for name, s in get_activation_tables(self.m.arch).items()
        ]
        import bass_rust as _bass_rust
        _bass_rust.insert_act_table_loads(self, tables)


def _mid_bcast(ap2d, count):
    """[P, F] -> [P, count, F] broadcasting along a new middle dim."""
    a = ap2d.ap
    assert len(a) == 2, a
    return bass.AP(tensor=ap2d.tensor, offset=ap2d.offset,
                   ap=[a[0], [0, count], a[1]])


def build(T_steps=T, dbg=False):
    nc = _Bacc("TRN2", target_bir_lowering=False, debug=False)

    def din(name, shape, dt=F32):
        return nc.dram_tensor(name, shape, dt, kind="ExternalInput").ap()

    xt = din("xt", [I, T_steps, BB], BF16)
    wx = din("wx", [I, NMB, 128], BF16)
    wgr = din("wgr", [I, NMB, 128], BF16)
    wgh = din("wgh", [128, NKH, NMB, 128], BF16)   # pre-halved
    wif2 = din("wif2", [128, NKH, 134], BF16)      # pre-halved
    bif2 = din("bif2", [1, 134], BF16)
    wout = din("wout", [128, 5, O], BF16)          # h chunks pre-halved
    bg = din("bg", [128, NMB])
    bout = din("bout", [O, 1])
    pert = din("pert", [128, 1])
    eyemb = din("eyemb", [128, 128], BF16)
    idnb = din("idnb", [128, 128], BF16)
    idnf = din("idnf", [128, 128])
    onesb = din("onesb", [1, 128], BF16)
    onesc = din("onesc", [128, 128], BF16)
    selb = din("selb", [BB, BB, 128], BF16)
    self_ = din("self_", [BB, BB, 128])
    alloc0 = din("alloc0", [128, BB])
    allocr0b = din("allocr0b", [128, 128], BF16)

    yT = nc.dram_tensor("yT", [O, T_steps, BB], F32, kind="ExternalOutput").ap()

    dbg_outs = {}
    if dbg:
        for nm, shp, dt in [
            ("d_h", [T_steps, 128, NKH, BB], BF16),     # holds 2h
            ("d_rv", [T_steps, W, BB], BF16),
            ("d_mem", [T_steps, 128, BB, W], BF16),
            ("d_link", [T_steps, 128, BB, 128], BF16),
            ("d_gates", [T_steps, 128, NMB, BB], F32),
            ("d_alloc", [T_steps, 128, BB], F32),
            ("d_ww", [T_steps, 128, BB], F32),
            ("d_usage", [T_steps, 128, BB], F32),
            ("d_wts", [T_steps, 128, BB], BF16),
            ("d_prec", [T_steps, 128, BB], F32),
        ]:
            dbg_outs[nm] = nc.dram_tensor(nm, shp, dt, kind="ExternalOutput").ap()

    with tile.TileContext(nc) as tc, ExitStack() as ctx:
        state = ctx.enter_context(tc.tile_pool(name="state", bufs=1))
        scr = ctx.enter_context(tc.tile_pool(name="scr", bufs=2))
        p_g = ctx.enter_context(tc.tile_pool(name="p_g", bufs=2, space="PSUM"))
        p_1 = ctx.enter_context(tc.tile_pool(name="p_1", bufs=1, space="PSUM"))

        def load(name, ap_dram, shape, dt=F32):
            t = state.tile(shape, dt, name=name)
            nc.sync.dma_start(out=t, in_=ap_dram)
            return t

        XT = load("XT", xt, [I, T_steps, BB], BF16)
        WX = load("WX", wx, [I, NMB, 128], BF16)
        WGR = load("WGR", wgr, [I, NMB, 128], BF16)
        WGH = load("WGH", wgh, [128, NKH, NMB, 128], BF16)
        WIF2 = load("WIF2", wif2, [128, NKH, 134], BF16)
        BIF2 = load("BIF2", bif2, [1, 134], BF16)
        WOUT = load("WOUT", wout, [128, 5, O], BF16)
        BG = load("BG", bg, [128, NMB])
        BOUT = load("BOUT", bout, [O, 1])
        PERT = load("PERT", pert, [128, 1])
        EYEMB = load("EYEMB", eyemb, [128, 128], BF16)
        IDNB = load("IDNB", idnb, [128, 128], BF16)
        IDNF = load("IDNF", idnf, [128, 128])
        ONESB = load("ONESB", onesb, [1, 128], BF16)
        ONESC = load("ONESC", onesc, [128, 128], BF16)
        SELB = load("SELB", selb, [BB, BB, 128], BF16)
        SELF = load("SELF", self_, [BB, BB, 128])
        ALLOC0 = load("ALLOC0", alloc0, [128, BB])
        ALLOCR0B = load("ALLOCR0B", allocr0b, [128, 128], BF16)

        # ---- persistent state ----
        GX = state.tile([128, NMB, T_steps, BB], F32, name="GX")
        H4 = state.tile([128, NKH, T_steps + 1, BB], BF16, name="H4")  # 2h
        RV = state.tile([W, T_steps + 1, BB], BF16, name="RV")
        MEM = state.tile([128, BB, W], BF16, name="MEM")
        LINKA = state.tile([128, BB, 128], BF16, name="LINKA")
        LINKB = state.tile([128, BB, 128], BF16, name="LINKB")
        USAGE = state.tile([128, BB], F32, name="USAGE")
        PREC = state.tile([128, BB], F32, name="PREC")
        CT = state.tile([128, NKH, BB], F32, name="CT")  # 2c
        RWA = state.tile([128, BB], BF16, name="RWA")
        RWB = state.tile([128, BB], BF16, name="RWB")

        for tl in (MEM, LINKA, LINKB, USAGE, PREC, CT, RWA, RWB):
            nc.vector.memset(tl, 0.0)
        nc.vector.memset(H4[:, :, 0, :], 0.0)
        nc.vector.memset(RV[:, 0, :], 0.0)

        # ---- pre-GEMM: GX[g,(t,b)] = W_ih[:, :64] @ x + (b_ih + b_hh) ----
        for mb in range(NMB):
            gx_ps = p_g.tile([128, T_steps * BB], F32, tag="gates", name="gx_ps")
            nc.tensor.matmul(gx_ps, WX[:, mb, :],
                             XT.rearrange("i t b -> i (t b)"),
                             start=True, stop=True)
            nc.vector.tensor_scalar(
                out=GX[:, mb, :, :].rearrange("p t b -> p (t b)"), in0=gx_ps,
                scalar1=BG[:, mb:mb + 1], scalar2=None, op0=OP.add)

        alloc_prev = ALLOC0
        allocrs_prev = _mid_bcast(ALLOCR0B, BB)
        g_ps_cur = p_g.tile([128, NMB, BB], F32, tag="gates", name="g_ps0")
        for mb in range(NMB):
            for c in range(1, 5):
                nc.tensor.matmul(g_ps_cur[:, mb, :], WGH[:, c - 1, mb, :],
                                 H4[:, c - 1, 0, :],
                                 start=(c == 1), stop=(c == 4))

        # =================== the scan ===================
        for t in range(T_steps):
            rw = RWA if t % 2 == 0 else RWB
            rw_new = RWB if t % 2 == 0 else RWA
            LINK = LINKA if t % 2 == 0 else LINKB
            LINKN = LINKB if t % 2 == 0 else LINKA

            # SMALL psum bank layout (f32 cols): 0:28 SC1P | 28:32 SWW |
            # 32:36 CSB | 36:40 WSB | 40:44 BWP | 44:48 RVP | 48:112 GRV |
            # 112:176 rwT(bf16 x2) | 176:240 wwT(bf16 x2) | 240:368 uT(f32)
            SMALL = p_1.tile([128, 432], F32, tag="small", name="SMALL")
            GRV = SMALL[:, 48:112].rearrange("p (m b) -> p m b", b=BB)

            # ---- 1. rw row-broadcast prep (rw known at step start) ----
            rwT_ps = SMALL[0:BB, 112:176].bitcast(BF16)
            nc.tensor.transpose(rwT_ps, rw, IDNB)
            rwTs = scr.tile([BB, 128], BF16, name="rwTs")
            nc.scalar.activation(out=rwTs, in_=rwT_ps, func=AF.Copy)
            RWBP = p_1.tile([128, BB, 128], F32, tag="rwbp", name="RWBP")
            for b in range(BB):
                nc.tensor.matmul(RWBP[:, b, :], SELB[:, b, :], rwTs,
                                 start=True, stop=True)
            RWBS = scr.tile([128, BB, 128], BF16, name="RWBS")
            nc.scalar.activation(out=RWBS, in_=RWBP, func=AF.Copy)

            # ---- 2. finish gates: + W_r @ rvec (h-part emitted in t-1) ----
            GRV = SMALL[:, 48:112].rearrange("p (m b) -> p m b", b=BB)
            g_ps = g_ps_cur
            for mb in range(NMB):
                nc.tensor.matmul(GRV[:, mb, :], WGR[:, mb, :], RV[:, t, :],
                                 start=True, stop=True)

            # ---- 3. LSTM cell (doubled c/h; no explicit sigmoids) ----
            gates = scr.tile([128, NMB, BB], F32, name="gates")
            nc.vector.tensor_add(gates, g_ps, GX[:, :, t, :])
            nc.vector.tensor_add(gates, gates, GRV)
            th = scr.tile([128, NMB, BB], F32, name="th")
            nc.scalar.activation(out=th, in_=gates, func=AF.Tanh, scale=0.5)
            ca = scr.tile([128, NKH, BB], F32, name="ca")
            nc.vector.scalar_tensor_tensor(        # (th_f+1)*2c
                out=ca, in0=th[:, 4:8, :], scalar=1.0, in1=CT,
                op0=OP.add, op1=OP.mult)
            cb = scr.tile([128, NKH, BB], F32, name="cb")
            nc.vector.scalar_tensor_tensor(        # (th_i+1)*tanh(g)
                out=cb, in0=th[:, 0:4, :], scalar=1.0, in1=th[:, 12:16, :],
                op0=OP.add, op1=OP.mult)
            nc.vector.scalar_tensor_tensor(        # 2c' = 0.5*ca + cb
                out=CT, in0=ca, scalar=0.5, in1=cb, op0=OP.mult, op1=OP.add)
            tanc = scr.tile([128, NKH, BB], F32, name="tanc")
            nc.scalar.activation(out=tanc, in_=CT, func=AF.Tanh, scale=0.5)
            nc.vector.scalar_tensor_tensor(        # 2h = (th_o+1)*tanh(c)
                out=H4[:, :, t + 1, :], in0=th[:, 8:12, :], scalar=1.0,
                in1=tanc, op0=OP.add, op1=OP.mult)

            # ---- 4. interface GEMM (transposed) + row-broadcasts ----
            ITFTP = p_1.tile([BB, 134], F32, tag="itftp", name="ITFTP")
            for c in range(NKH):
                nc.tensor.matmul(ITFTP, H4[:, c, t + 1, :], WIF2[:, c, :],
                                 start=(c == 0), stop=False)
            nc.tensor.matmul(ITFTP, ONESB[:, 0:BB], BIF2,
                             start=False, stop=True)
            itfTs = scr.tile([BB, 136], BF16, name="itfTs")
            nc.scalar.activation(out=itfTs[:, 0:134], in_=ITFTP, func=AF.Copy)
            # kn2 -> itfTs col 134 (bf16 ok; feeds rsqrt)
            ksq = scr.tile([BB, W], F32, name="ksq")
            knT = scr.tile([BB, 1], F32, name="knT")
            nc.scalar.activation(out=ksq, in_=itfTs[:, 0:64], func=AF.Square,
                                 accum_out=knT)
            nc.vector.tensor_copy(itfTs[:, 134:135], knT)

            EWKP = p_1.tile([128, BB, 128], F32, tag="ewkp", name="EWKP")
            SC1P = SMALL[:, 0:28].rearrange("p (b s) -> p b s", s=7)
            for b in range(BB):
                nc.tensor.matmul(EWKP[:, b, :], SELB[:, b, :],
                                 itfTs[:, 0:128], start=True, stop=True)
                nc.tensor.matmul(SMALL[:, b * 7:b * 7 + 7], SELB[:, b, :],
                                 itfTs[:, 128:135], start=True, stop=True)
            EWVS = scr.tile([128, BB, W], BF16, name="EWVS")
            nc.scalar.activation(out=EWVS, in_=EWKP[:, :, 64:128],
                                 func=AF.Copy)
            EKYS = scr.tile([128, BB, W], BF16, name="EKYS")
            nc.scalar.activation(out=EKYS, in_=EWKP[:, :, 0:64], func=AF.Copy)

            # ---- 5. interface activations ----
            the = scr.tile([128, BB, W], BF16, name="the")   # tanh(e/2)
            nc.scalar.activation(out=the, in_=EWKP[:, :, 0:64],
                                 func=AF.Tanh, scale=0.5)
            th2 = scr.tile([128, BB, 2], F32, name="th2")
            nc.scalar.activation(out=th2, in_=SC1P[:, :, 0:2],
                                 func=AF.Tanh, scale=0.5)
            t1w = scr.tile([128, BB], F32, name="t1w")
            nc.vector.tensor_scalar(out=t1w, in0=th2[:, :, 0], scalar1=1.0,
                                    scalar2=None, op0=OP.add)
            wgag4 = scr.tile([128, BB], F32, name="wgag4")   # 4*wg*ag
            nc.vector.scalar_tensor_tensor(
                out=wgag4, in0=th2[:, :, 1], scalar=1.0, in1=t1w,
                op0=OP.add, op1=OP.mult)
            ex3 = scr.tile([128, BB, 3], F32, name="ex3")    # unnormalized rmode
            nc.scalar.activation(out=ex3, in_=SC1P[:, :, 3:6], func=AF.Exp)
            # rstr2 = log2(1 + e^z)  (bit-hack log2)
            ez = scr.tile([128, BB], F32, name="ez")
            nc.scalar.activation(out=ez, in_=SC1P[:, :, 2], func=AF.Exp)
            basef = scr.tile([128, BB], F32, name="basef")
            nc.vector.tensor_scalar(out=basef, in0=ez, scalar1=1.0,
                                    scalar2=None, op0=OP.add)
            mi1 = scr.tile([128, BB], I32, name="mi1")
            nc.vector.scalar_tensor_tensor(
                out=mi1, in0=basef.bitcast(I32), scalar=0.0,
                in1=MAND.broadcast_to([128, BB]),
                op0=OP.bypass, op1=OP.bitwise_and)
            mi = scr.tile([128, BB], I32, name="mi")
            nc.vector.scalar_tensor_tensor(
                out=mi, in0=mi1, scalar=0.0,
                in1=MOR.broadcast_to([128, BB]),
                op0=OP.bypass, op1=OP.bitwise_or)
            ei = scr.tile([128, BB], I32, name="ei")
            nc.vector.scalar_tensor_tensor(
                out=ei, in0=basef.bitcast(I32), scalar=0.0,
                in1=SH23.broadcast_to([128, BB]),
                op0=OP.bypass, op1=OP.logical_shift_right)
            mf = mi.bitcast(F32)
            h1 = scr.tile([128, BB], F32, name="h1")
            nc.vector.tensor_scalar(out=h1, in0=mf, scalar1=C2Q, scalar2=C1Q,
                                    op0=OP.mult, op1=OP.add)
            h2a = scr.tile([128, BB], F32, name="h2a")
            nc.vector.tensor_mul(h2a, h1, mf)
            rstr2 = scr.tile([128, BB], F32, name="rstr2")
            nc.vector.scalar_tensor_tensor(
                out=rstr2, in0=ei, scalar=(C0Q - 127.0), in1=h2a,
                op0=OP.add, op1=OP.add)

            # ---- 6. ww and friends ----
            ww = scr.tile([128, BB], F32, name="ww")
            nc.vector.tensor_mul(ww, wgag4, alloc_prev)
            wwb = scr.tile([128, BB], BF16, name="wwb")
            nc.scalar.activation(out=wwb, in_=ww, func=AF.Copy)
            onemw = scr.tile([128, BB], F32, name="onemw")
            nc.vector.tensor_scalar(out=onemw, in0=ww, scalar1=-1.0,
                                    scalar2=1.0, op0=OP.mult, op1=OP.add)

            # ---- 7. memory write:  mem -= ww*(0.5*(the+1)*mem - wvec) ----
            q1 = scr.tile([128, BB, W], BF16, name="q1")
            nc.vector.scalar_tensor_tensor(
                out=q1, in0=the, scalar=1.0, in1=MEM,
                op0=OP.add, op1=OP.mult)
            r1 = scr.tile([128, BB, W], BF16, name="r1")
            nc.vector.scalar_tensor_tensor(
                out=r1, in0=q1, scalar=0.5, in1=EWVS,
                op0=OP.mult, op1=OP.subtract)
            s1 = scr.tile([128, BB, W], BF16, name="s1")
            nc.vector.tensor_tensor(
                s1, r1, wwb.broadcast_to([128, BB, W]), OP.mult)
            nc.vector.tensor_sub(MEM, MEM, s1)

            # ---- 8. usage update: u' = 1 - (1-u)(1-ww) ----
            qus = scr.tile([128, BB], F32, name="qus")
            nc.vector.scalar_tensor_tensor(
                out=qus, in0=USAGE, scalar=1.0, in1=onemw,
                op0=OP.subtract, op1=OP.mult)   # -(1-u)(1-ww)
            nc.vector.tensor_scalar(out=USAGE, in0=qus, scalar1=1.0,
                                    scalar2=None, op0=OP.add)

            # ---- 10. link update (bf16) + precedence ----
            # ww_j rows come from the alloc row-broadcast (prev step) * wgag4
            WWRPS = scr.tile([128, BB, 128], BF16, name="WWRPS")
            for b in range(BB):
                nc.vector.tensor_scalar(
                    out=WWRPS[:, b, :], in0=allocrs_prev[:, b, :],
                    scalar1=wgag4[:, b:b + 1], scalar2=None, op0=OP.mult)
            negld = scr.tile([128, BB, 128], BF16, name="negld")
            t2m = scr.tile([128, BB, 128], BF16, name="t2m")
            for b in range(BB):
                nc.vector.scalar_tensor_tensor(
                    out=negld[:, b, :], in0=WWRPS[:, b, :],
                    scalar=onemw[:, b:b + 1], in1=LINK[:, b, :],
                    op0=OP.subtract, op1=OP.mult)
                nc.vector.scalar_tensor_tensor(
                    out=t2m[:, b, :], in0=WWRPS[:, b, :],
                    scalar=PREC[:, b:b + 1], in1=EYEMB,
                    op0=OP.mult, op1=OP.mult)
            nc.vector.tensor_tensor(LINKN, t2m, negld, OP.subtract)
            # prec' = (1 - sum_i ww) * prec + ww   (colsum via ones-matmul)
            SWWP = SMALL[:, 28:32]
            nc.tensor.matmul(SWWP, ONESC, wwb, start=True, stop=True)
            oms = scr.tile([128, BB], F32, name="oms")
            nc.vector.tensor_scalar(out=oms, in0=SWWP, scalar1=-1.0,
                                    scalar2=1.0, op0=OP.mult, op1=OP.add)
            nc.vector.tensor_mul(PREC, PREC, oms)
            nc.vector.tensor_add(PREC, PREC, ww)

            # ---- 11. read ----
            BWP = SMALL[:, 40:44]
            for b in range(BB):
                nc.tensor.matmul(BWP[:, b:b + 1], LINKN[:, b, :],
                                 rw[:, b:b + 1], start=True, stop=True)
            fw = scr.tile([128, BB], F32, name="fw")
            junkF = scr.tile([128, 128], BF16, name="junkF")
            for b in range(BB):
                nc.vector.scalar_tensor_tensor(
                    out=junkF, in0=LINKN[:, b, :], scalar=1.0,
                    in1=RWBS[:, b, :], op0=OP.mult, op1=OP.mult,
                    accum_out=fw[:, b:b + 1])
            dsq = scr.tile([128, BB, W], BF16, name="dsq")
            nc.vector.tensor_mul(dsq, MEM, EKYS)
            dotv = scr.tile([128, BB], F32, name="dotv")
            nc.vector.tensor_reduce(out=dotv, in_=dsq, axis=AX.X, op=OP.add)
            msq = scr.tile([128, BB, W], F32, name="msq")
            nc.scalar.activation(out=msq, in_=MEM, func=AF.Square)
            mn2 = scr.tile([128, BB], F32, name="mn2")
            nc.vector.tensor_reduce(out=mn2, in_=msq, axis=AX.X, op=OP.add)
            # rq = rsqrt(kn2*mn2 + eps)  (bit-hack + 1 Newton)
            p2 = scr.tile([128, BB], F32, name="p2")
            nc.vector.scalar_tensor_tensor(
                out=p2, in0=SC1P[:, :, 6], scalar=1e-16, in1=mn2,
                op0=OP.max, op1=OP.mult)
            ti = scr.tile([128, BB], I32, name="ti")
            nc.vector.scalar_tensor_tensor(
                out=ti, in0=p2.bitcast(I32), scalar=0.0,
                in1=SH1.broadcast_to([128, BB]),
                op0=OP.bypass, op1=OP.logical_shift_right)
            y0i = scr.tile([128, BB], I32, name="y0i")
            nc.vector.scalar_tensor_tensor(
                out=y0i, in0=ti, scalar=-1.0,
                in1=KMAGIC.broadcast_to([128, BB]), op0=OP.mult, op1=OP.add)
            y0 = y0i.bitcast(F32)
            ysq = scr.tile([128, BB], F32, name="ysq")
            nc.vector.tensor_mul(ysq, y0, y0)
            t2n = scr.tile([128, BB], F32, name="t2n")
            nc.vector.tensor_mul(t2n, ysq, p2)
            fnr = scr.tile([128, BB], F32, name="fnr")
            nc.vector.tensor_scalar(out=fnr, in0=t2n, scalar1=-0.5,
                                    scalar2=1.5, op0=OP.mult, op1=OP.add)
            rq = scr.tile([128, BB], F32, name="rq")
            nc.vector.tensor_mul(rq, fnr, y0)
            cos2 = scr.tile([128, BB], F32, name="cos2")
            nc.vector.tensor_mul(cos2, dotv, rq)
            expv2 = scr.tile([128, BB], F32, name="expv2")
            nc.vector.tensor_mul(expv2, cos2, rstr2)
            cwn16 = scr.tile([128, BB], BF16, name="cwn16")
            nc.scalar.activation(out=cwn16, in_=expv2, func=AF.Exp, scale=LN2)
            CSBP = SMALL[:, 32:36]
            nc.tensor.matmul(CSBP, ONESC, cwn16, start=True, stop=True)
            csbr = scr.tile([128, BB], F32, name="csbr")
            nc.vector.reciprocal_approx_fast(out=csbr, in_=CSBP)
            cwf = scr.tile([128, BB], F32, name="cwf")
            nc.vector.tensor_mul(cwf, cwn16, csbr)
            cw2 = scr.tile([128, BB], F32, name="cw2")
            nc.vector.tensor_mul(cw2, cwf, ex3[:, :, 2])
            bwf = scr.tile([128, BB], F32, name="bwf")
            nc.vector.tensor_mul(bwf, BWP, ex3[:, :, 0])
            fwf = scr.tile([128, BB], F32, name="fwf")
            nc.vector.tensor_mul(fwf, fw, ex3[:, :, 1])
            wtsa = scr.tile([128, BB], F32, name="wtsa")
            nc.vector.scalar_tensor_tensor(
                out=wtsa, in0=bwf, scalar=1e-8, in1=fwf,
                op0=OP.add, op1=OP.add)
            wtsu16 = scr.tile([128, BB], BF16, name="wtsu16")
            nc.vector.tensor_add(wtsu16, wtsa, cw2)
            WSBP = SMALL[:, 36:40]
            nc.tensor.matmul(WSBP, ONESC, wtsu16, start=True, stop=True)
            wsbr = scr.tile([128, BB], F32, name="wsbr")
            nc.vector.reciprocal_approx_fast(out=wsbr, in_=WSBP)
            nc.vector.tensor_mul(rw_new, wtsu16, wsbr)
            RVP = SMALL[0:W, 44:48]
            for b in range(BB):
                nc.tensor.matmul(RVP[:, b:b + 1], MEM[:, b, :],
                                 wtsu16[:, b:b + 1], start=True, stop=True)
            nc.vector.tensor_mul(RV[:, t + 1, :], RVP, wsbr[0:W, :])

            # ---- 9. allocation chain for step t+1 ----
            if t + 1 < T_steps:
                ucol = scr.tile([128, BB], F32, name="ucol")
                nc.vector.tensor_scalar(out=ucol, in0=USAGE, scalar1=PERT,
                                        scalar2=None, op0=OP.add)
                uT_ps = SMALL[0:BB, 176:304]
                nc.tensor.transpose(uT_ps, ucol, IDNF)
                uTs = scr.tile([BB, 128], F32, name="uTs")
                nc.scalar.activation(out=uTs, in_=uT_ps, func=AF.Copy)
                UBC = p_1.tile([128, BB, 128], F32, tag="ubc", name="UBC")
                for b in range(BB):
                    nc.tensor.matmul(UBC[:, b, :], SELF[:, b, :], uTs,
                                     start=True, stop=True)
                UBCS = scr.tile([128, BB, 128], F32, name="UBCS")
                nc.scalar.activation(out=UBCS, in_=UBC, func=AF.Copy)
                # factor = max([u~_j >= u~_i], u~_j): 1 for excluded j
                # (u~ <= 1+1.3e-5), u~_j for included j — no min pass needed
                m4 = scr.tile([128, BB, 128], F32, name="m4")
                for b in range(BB):
                    nc.vector.scalar_tensor_tensor(
                        out=m4[:, b, :], in0=UBCS[:, b, :],
                        scalar=ucol[:, b:b + 1], in1=UBCS[:, b, :],
                        op0=OP.is_ge, op1=OP.max)
                pprod = scr.tile([128, BB], F32, name="pprod")
                nc.vector.tensor_reduce(out=pprod, in_=m4, axis=AX.X,
                                        op=OP.mult)
                onemu2 = scr.tile([128, BB], F32, name="onemu2")
                nc.vector.tensor_scalar(out=onemu2, in0=USAGE, scalar1=-1.0,
                                        scalar2=1.0, op0=OP.mult, op1=OP.add)
                alloc_new = scr.tile([128, BB], F32, name="alloc_new")
                nc.vector.tensor_mul(alloc_new, onemu2, pprod)
            else:
                alloc_new = alloc_prev


            if dbg:
                nc.sync.dma_start(out=dbg_outs["d_gates"][t], in_=gates)
                nc.sync.dma_start(out=dbg_outs["d_alloc"][t], in_=alloc_prev)
                nc.sync.dma_start(out=dbg_outs["d_ww"][t], in_=ww)
                nc.sync.dma_start(out=dbg_outs["d_usage"][t], in_=USAGE)
                nc.sync.dma_start(out=dbg_outs["d_mem"][t], in_=MEM)
                nc.sync.dma_start(out=dbg_outs["d_link"][t], in_=LINKN)
                nc.sync.dma_start(out=dbg_outs["d_wts"][t], in_=rw_new)
                nc.sync.dma_start(out=dbg_outs["d_h"][t],
                                  in_=H4[:, :, t + 1, :])
                nc.sync.dma_start(out=dbg_outs["d_rv"][t], in_=RV[:, t + 1, :])
                nc.sync.dma_start(out=dbg_outs["d_prec"][t], in_=PREC)

            alloc_prev = alloc_new
            allocrs_prev = allocrs_new

            # ---- 12. next step's h-part gate GEMM (fills PE idle time) ----
            if t + 1 < T_steps:
                g_ps_cur = p_g.tile([128, NMB, BB], F32, tag="gates",
                                    name="g_ps_n")
                for mb in range(NMB):
                    for c in range(1, 5):
                        nc.tensor.matmul(g_ps_cur[:, mb, :],
                                         WGH[:, c - 1, mb, :],
                                         H4[:, c - 1, t + 1, :],
                                         start=(c == 1), stop=(c == 4))

        # ---- post-GEMM: y = W_out @ [h; rvec] + b_out ----
        y_ps = p_g.tile([O, T_steps * BB], F32, tag="gates", name="y_ps")
        nc.tensor.matmul(y_ps, WOUT[0:W, 0, :],
                         RV[:, 1:T_steps + 1, :].rearrange("p t b -> p (t b)"),
                         start=True, stop=False)
        for c in range(1, 5):
            rhs = H4[:, c - 1, 1:T_steps + 1, :].rearrange("p t b -> p (t b)")
            nc.tensor.matmul(y_ps, WOUT[:, c, :], rhs,
                             start=False, stop=(c == 4))
        y_sb = state.tile([O, T_steps * BB], F32, name="y_sb")
        nc.vector.tensor_scalar(out=y_sb, in0=y_ps, scalar1=BOUT,
                                scalar2=None, op0=OP.add)
        nc.sync.dma_start(out=yT, in_=y_sb.rearrange("o (t b) -> o t b", b=BB))

    nc.finalize()
    return nc


def prep_inputs(x, W_ih, W_hh, b_ih, b_hh, W_if, b_if, W_out, b_out,
                T_steps=T):
    """Host-side layout packing of full inputs into per-core in_maps."""
    x = np.asarray(x, np.float32)
    W_ih = np.asarray(W_ih, np.float32)
    W_hh = np.asarray(W_hh, np.float32)
    W_if_ = np.asarray(W_if, np.float32)
    W_out_ = np.asarray(W_out, np.float32)
    b_ih = np.asarray(b_ih, np.float32)
    b_hh = np.asarray(b_hh, np.float32)
    b_if_ = np.asarray(b_if, np.float32)
    b_out_ = np.asarray(b_out, np.float32)

    # gate-dim m-block permutation: source order i,f,g,o -> kernel i,f,o,g
    gperm = [0, 1, 2, 3, 4, 5, 6, 7, 12, 13, 14, 15, 8, 9, 10, 11]
    # g-gate rows are pre-doubled so one tanh(x/2) serves every block
    gscale = np.ones((NMB, 1, 1), np.float32)
    gscale[12:16] = 2.0
    wx = np.ascontiguousarray(
        W_ih[:, :64].T.reshape(I, NMB, 128)[:, gperm, :]
        * gscale.transpose(1, 0, 2))
    wgr = np.ascontiguousarray(
        W_ih[:, 64:128].T.reshape(I, NMB, 128)[:, gperm, :]
        * gscale.transpose(1, 0, 2))
    # W_hh halved: h is stored doubled
    wgh = np.ascontiguousarray(
        (0.5 * W_hh).T.reshape(NKH, 128, NMB, 128).transpose(1, 0, 2, 3)[
            :, :, gperm, :] * gscale.reshape(1, 1, NMB, 1))
    # packed interface dims: [0..129, 194..197] -> 134; W_if halved
    pdims = list(range(130)) + [194, 195, 196, 197]
    wifT = (0.5 * W_if_).T.reshape(NKH, 128, IF).transpose(1, 0, 2)
    wif2 = np.ascontiguousarray(wifT[:, :, pdims])
    bif2 = np.ascontiguousarray(b_if_[pdims].reshape(1, 134))
    wout = np.zeros((128, 5, O), np.float32)
    wout[0:W, 0, :] = W_out_[:, 512:576].T          # rvec part: real scale
    wout[:, 1:5, :] = (0.5 * W_out_[:, 0:512]).T.reshape(
        NKH, 128, O).transpose(1, 0, 2)
    bgv = np.ascontiguousarray(
        ((b_ih + b_hh).reshape(NMB, 128)[gperm, :] * gscale[:, :, 0]).T)
    boutv = np.ascontiguousarray(b_out_.reshape(O, 1))

    a0 = np.zeros((128, BB), np.float32)
    a0[0, :] = 0.25           # alloc/4
    ar0 = np.zeros((128, 128), np.float32)
    ar0[:, 0] = 0.25

    common = {
        "wx": wx.astype(BFNP), "wgr": wgr.astype(BFNP),
        "wgh": wgh.astype(BFNP), "wif2": wif2.astype(BFNP),
        "bif2": bif2.astype(BFNP),
        "wout": wout.astype(BFNP),
        "bg": bgv, "bout": boutv,
        "pert": (np.arange(128, dtype=np.float32) * 1e-7).reshape(128, 1),
        "eyemb": (1.0 - np.eye(128)).astype(BFNP),
        "idnb": np.eye(128, dtype=BFNP),
        "idnf": np.eye(128, dtype=np.float32),
        "onesb": np.ones((1, 128), BFNP),
        "onesc": np.ones((128, 128), BFNP),
        "selb": np.ascontiguousarray(
            np.eye(BB, dtype=BFNP)[:, :, None]
            * np.ones((1, 1, 128), BFNP)),
        "self_": np.ascontiguousarray(
            np.eye(BB, dtype=np.float32)[:, :, None]
            * np.ones((1, 1, 128), np.float32)),
        "alloc0": a0,
        "allocr0b": ar0.astype(BFNP),
    }
    in_maps = []
    for core in range(NCORES):
        xc = x[core * BB:(core + 1) * BB, :T_steps, :]
        m = dict(common)
        m["xt"] = np.ascontiguousarray(xc.transpose(2, 1, 0)).astype(BFNP)
        in_maps.append(m)
    return in_maps


_NC_CACHE = {}


def run(inputs, T_steps=T, dbg=False, trace=False):
    key = (T_steps, dbg)
    if key not in _NC_CACHE:
        _NC_CACHE[key] = build(T_steps, dbg)
    nc = _NC_CACHE[key]
    in_maps = prep_inputs(**inputs, T_steps=T_steps)
    res = bass_utils.run_bass_kernel_spmd(
        nc, in_maps, core_ids=list(range(NCORES)), trace=trace)
    outs = [np.asarray(r["yT"]).transpose(2, 1, 0) for r in res.results]
    return np.concatenate(outs, axis=0), res


def kernel(**inputs):
    y, _ = run(inputs, T_steps=T, dbg=False)
    return np.ascontiguousarray(y).astype(np.float32)
